# revision 24
# baseline (speedup 1.0000x reference)
"""Multi-head attention (N=2, L=2048, 16 heads x 64) on 8 TRN2 NeuronCores.

Head-parallel attention (2 heads/core) with a per-batch software pipeline:
attention emission interleaved between projection blocks (so the shared
PSUM ring rotates in data-readiness order), per-batch AllToAll (head-split
-> sequence-split) with collective #0 hidden under batch-1 compute and the
batch-0 output projection hidden under collective #1.

Key scheduling choices:
- Few, large DMAs: weights host-pre-shuffled to partition-major [128,t,d]
  so each const load is one contiguous chunk per partition; input quarters
  loaded as [128,8,512] tiles; issue split across both HWDGE rings.
- Same-bank PSUM accumulation stalls (~+100ns/matmul) avoided by
  interleaving independent chain pairs (K/V and V/V projection chains, the
  two heads' AV chains, output-projection mt-groups) across banks.
- Softmax normalization without DMA round trips: ones-column in V gives the
  denominator as AV row 64; K=1 matmul broadcasts it, reciprocal_approx_fast
  (5x faster than DVE reciprocal) + one DVE multiply normalize.
- exp is one ScalarE call per k-tile covering both heads via a 3D AP.
"""
import sys

sys.path.insert(0, "/opt/trn_rl_repo")

import numpy as np
import ml_dtypes

import concourse.bass as bass
import concourse.bacc as bacc
import concourse.mybir as mybir
import concourse.tile as tile
from concourse.bass_utils import run_bass_kernel_spmd

BF16 = ml_dtypes.bfloat16

DM = 1024      # dmodel
DK = 64        # head dim
H = 16         # heads
NB = 2         # batch
L = 2048       # seq len
R = NB * L
NC = 8         # cores
HPC = H // NC  # heads per core = 2
DPC = HPC * DK  # depth per core = 128

SW = 512       # q sub-window
KT = 128       # k tile
NQS = L // SW   # 4 q blocks per batch
NKT = L // KT   # 16 k tiles per batch
CB = L // NC    # 256: per-batch per-core output chunk
VW = 65 * HPC   # 130: augmented v width (both heads, +ones col each)

_CACHE = {}


def _classify_blocks(mask):
    """Per (qs, kt) block: 0=skip, 1=full, 2=partial (+ q-span, pattern)."""
    mask = np.asarray(mask, dtype=bool)
    cls = [[0] * NKT for _ in range(NQS)]
    span = [[None] * NKT for _ in range(NQS)]
    pat_ids = {}
    pats = []
    pat_idx = [[-1] * NKT for _ in range(NQS)]
    for qs in range(NQS):
        for kt in range(NKT):
            sub = mask[qs * SW:(qs + 1) * SW, kt * KT:(kt + 1) * KT]
            rows = np.nonzero(sub.any(axis=1))[0]
            if rows.size == 0:
                cls[qs][kt] = 0
            elif sub.all():
                cls[qs][kt] = 1
                span[qs][kt] = (0, SW)
            else:
                cls[qs][kt] = 2
                span[qs][kt] = (int(rows[0]), int(rows[-1]) + 1)
                pat = np.ascontiguousarray(sub.T).astype(BF16)  # [128 k, SW q]
                key = pat.tobytes()
                if key not in pat_ids:
                    pat_ids[key] = len(pats)
                    pats.append(pat)
                pat_idx[qs][kt] = pat_ids[key]
    # the first included kt of each sub-window must cover the full 512
    # columns (its start=True matmul clears PSUM has_written)
    for qs in range(NQS):
        for kt in range(NKT):
            if cls[qs][kt]:
                span[qs][kt] = (0, SW)
                break
    if not pats:
        pats.append(np.ones((KT, SW), dtype=BF16))
    return cls, span, pat_idx, np.stack(pats)


def _build(cls_, span_, pidx, n_pat):
    nc = bacc.Bacc("TRN2", target_bir_lowering=False, debug=False,
                   enable_asserts=False, num_devices=NC)
    f32, bf16 = mybir.dt.float32, mybir.dt.bfloat16
    EXP = mybir.ActivationFunctionType.Exp
    MUL = mybir.AluOpType.mult

    # weights arrive host-pre-shuffled into partition-major layouts so every
    # const DMA is one contiguous chunk per partition (few, large descriptors)
    xtb = nc.dram_tensor("xtb", [DM, R], bf16, kind="ExternalInput")
    ytb = nc.dram_tensor("ytb", [DM, R], bf16, kind="ExternalInput")
    wq = nc.dram_tensor("wq", [128, 8, DPC], bf16, kind="ExternalInput")
    wk = nc.dram_tensor("wk", [128, 8, DPC], bf16, kind="ExternalInput")
    wv = nc.dram_tensor("wv", [128, 8, VW], bf16, kind="ExternalInput")
    wo = nc.dram_tensor("wo", [128, 8, DM], bf16, kind="ExternalInput")
    bqd = nc.dram_tensor("bq", [DPC, 1], f32, kind="ExternalInput")
    bkd = nc.dram_tensor("bk", [DPC, 1], f32, kind="ExternalInput")
    bv1 = nc.dram_tensor("bv1", [1, VW], bf16, kind="ExternalInput")
    bod = nc.dram_tensor("bo", [128, 8, 1], f32, kind="ExternalInput")
    mpat = nc.dram_tensor("mpat", [KT, n_pat, SW], bf16, kind="ExternalInput")
    out_t = nc.dram_tensor("out_t", [DM, NB * CB], f32, kind="ExternalOutput")

    with tile.TileContext(nc) as tc:
        with (
            tc.tile_pool(name="cst", bufs=1) as cst,
            tc.tile_pool(name="xy", bufs=6) as xy,
            tc.tile_pool(name="big", bufs=1) as big,
            tc.tile_pool(name="expp", bufs=12) as expp,
            tc.tile_pool(name="nrm", bufs=3) as nrm,
            tc.tile_pool(name="wos", bufs=2) as wos,
            tc.tile_pool(name="osb", bufs=3) as osb,
            tc.tile_pool(name="sp", bufs=3, space="PSUM") as sp,
            tc.tile_pool(name="avp", bufs=2, space="PSUM") as avp,
            tc.tile_pool(name="dram", bufs=1, space="DRAM") as dram,
        ):
            # ---- constants (scalar HWDGE ring; contiguous partition-major) ----
            bq_sb = cst.tile([DPC, 1], f32)
            bk_sb = cst.tile([DPC, 1], f32)
            bv1_sb = cst.tile([1, VW], bf16)
            bo_sb = cst.tile([128, 8, 1], f32)
            nc.scalar.dma_start(bk_sb[:], bkd[:])
            nc.scalar.dma_start(bq_sb[:], bqd[:])
            nc.scalar.dma_start(bv1_sb[:], bv1[:])
            nc.scalar.dma_start(bo_sb[:], bod[:, :, :])
            wq_sb = cst.tile([128, 8, DPC], bf16)
            wk_sb = cst.tile([128, 8, DPC], bf16)
            wv_sb = cst.tile([128, 8, VW], bf16)
            wo_sb = cst.tile([128, 8, DM], bf16)
            nc.scalar.dma_start(wk_sb[:], wk[:, :, :])
            nc.scalar.dma_start(wv_sb[:], wv[:, :, :])
            nc.scalar.dma_start(wq_sb[:], wq[:, :, :])
            mpat_sb = cst.tile([KT, n_pat, SW], bf16)
            nc.scalar.dma_start(mpat_sb[:], mpat[:, :, :])
            ones_row = cst.tile([1, 128], bf16)
            nc.vector.memset(ones_row[:], 1.0)
            ones65 = cst.tile([65, DK], bf16)
            nc.vector.memset(ones65[:], 1.0)

            # preload the exp table set during the DMA phase
            bar_sb = cst.tile([1, 8], f32)
            nc.vector.memset(bar_sb[:], 0.0)
            dum = cst.tile([1, 8], f32)
            nc.scalar.activation(dum[:], bar_sb[:], EXP)

            # ---- start-of-kernel barrier (absorbs launch skew) ----
            bar_in = dram.tile([1, 8], f32, tag="bar_in")
            bar_out = dram.tile([1, 8], f32, tag="bar_out")
            nc.sync.dma_start(bar_in[:], bar_sb[:])
            nc.gpsimd.collective_compute(
                "AllReduce", mybir.AluOpType.add,
                replica_groups=[list(range(NC))],
                ins=[bar_in.opt()], outs=[bar_out.opt()])

            qT = [big.tile([DPC, L], bf16, tag=f"qT{n}", name=f"qT{n}") for n in range(NB)]
            kT = [big.tile([DPC, L], bf16, tag=f"kT{n}", name=f"kT{n}") for n in range(NB)]
            vaug = [big.tile([128, NKT * VW], bf16, tag=f"va{n}", name=f"va{n}") for n in range(NB)]
            headT = [[big.tile([DK, L], bf16, tag=f"hT{n}{hp}", name=f"hT{n}{hp}")
                      for hp in range(HPC)] for n in range(NB)]

            a2a_in = [dram.tile([NC, DPC, CB], bf16, tag=f"a2ai{n}", name=f"a2ai{n}")
                      for n in range(NB)]
            a2a_out = [dram.tile([NC, DPC, CB], bf16, tag=f"a2ao{n}", name=f"a2ao{n}")
                       for n in range(NB)]

            ytile = [[None] * NQS for _ in range(NB)]
            xtile = [[None] * NQS for _ in range(NB)]

            def emit_inputs(n):
                for b in range(NQS):
                    yt = xy.tile([128, 8, SW], bf16, tag="xy", name=f"y{n}b{b}")
                    nc.sync.dma_start(
                        yt[:], ytb[:, n * L + b * SW:n * L + (b + 1) * SW]
                        .rearrange("(t p) c -> p t c", p=128))
                    ytile[n][b] = yt
                    xt = xy.tile([128, 8, SW], bf16, tag="xy", name=f"x{n}b{b}")
                    nc.sync.dma_start(
                        xt[:], xtb[:, n * L + b * SW:n * L + (b + 1) * SW]
                        .rearrange("(t p) c -> p t c", p=128))
                    xtile[n][b] = xt

            def _v_chain(n, b, j, psv, dt, last):
                # one step of a V-projection chain (stationary = y k-tile)
                yt = ytile[n][b]
                if dt < 8:
                    nc.tensor.matmul(psv[:, :VW], yt[:, dt, j * KT:(j + 1) * KT],
                                     wv_sb[:, dt, :], start=(dt == 0), stop=False)
                else:
                    nc.tensor.matmul(psv[:, :VW], ones_row[:], bv1_sb[:],
                                     start=False, stop=True)

            def emit_proj_block(n, b):
                # chains interleaved in PAIRS so consecutive matmuls hit
                # alternating PSUM banks (avoids same-bank drain stalls)
                yt = ytile[n][b]
                xt = xtile[n][b]
                # pair 1: K chain & V chain j=0
                ps_k = sp.tile([128, 1024], f32, tag="sp", name=f"kp{n}{b}")
                psv0 = sp.tile([128, 1024], f32, tag="sp", name=f"vp{n}{b}0")
                for dt in range(9):
                    if dt < 8:
                        nc.tensor.matmul(ps_k[:DPC, :SW], wk_sb[:, dt, :], yt[:, dt, :],
                                         start=(dt == 0), stop=(dt == 7))
                    _v_chain(n, b, 0, psv0, dt, False)
                nc.vector.tensor_scalar_add(kT[n][:, b * SW:(b + 1) * SW],
                                            ps_k[:DPC, :SW], bk_sb[:])
                nc.vector.tensor_copy(vaug[n][:, (b * 4) * VW:(b * 4 + 1) * VW],
                                      psv0[:, :VW])
                # pair 2: V chains j=1 & j=2
                psv1 = sp.tile([128, 1024], f32, tag="sp", name=f"vp{n}{b}1")
                psv2 = sp.tile([128, 1024], f32, tag="sp", name=f"vp{n}{b}2")
                for dt in range(9):
                    _v_chain(n, b, 1, psv1, dt, False)
                    _v_chain(n, b, 2, psv2, dt, False)
                nc.vector.tensor_copy(vaug[n][:, (b * 4 + 1) * VW:(b * 4 + 2) * VW],
                                      psv1[:, :VW])
                nc.vector.tensor_copy(vaug[n][:, (b * 4 + 2) * VW:(b * 4 + 3) * VW],
                                      psv2[:, :VW])
                # pair 3: V chain j=3 & Q chain
                psv3 = sp.tile([128, 1024], f32, tag="sp", name=f"vp{n}{b}3")
                ps_q = sp.tile([128, 1024], f32, tag="sp", name=f"qp{n}{b}")
                for dt in range(9):
                    _v_chain(n, b, 3, psv3, dt, False)
                    if dt < 8:
                        nc.tensor.matmul(ps_q[:DPC, :SW], wq_sb[:, dt, :], xt[:, dt, :],
                                         start=(dt == 0), stop=(dt == 7))
                nc.vector.tensor_copy(vaug[n][:, (b * 4 + 3) * VW:(b * 4 + 4) * VW],
                                      psv3[:, :VW])
                nc.vector.tensor_scalar_add(qT[n][:, b * SW:(b + 1) * SW],
                                            ps_q[:DPC, :SW], bq_sb[:])

            def emit_attn_qs(n, qs):
                kts = [kt for kt in range(NKT) if cls_[qs][kt]]
                exp_tiles = {}
                avs = [avp.tile([65, SW], f32, tag="avp", name=f"av{n}{qs}{hp}")
                       for hp in range(HPC)]

                def emit_scores(kt):
                    a, b = span_[qs][kt]
                    ps = sp.tile([128, 1024], f32, tag="sp", name=f"s{n}{qs}{kt}")
                    for hp in range(HPC):
                        hs = hp * DK
                        nc.tensor.matmul(
                            ps[:KT, hp * SW + a:hp * SW + b],
                            kT[n][hs:hs + DK, kt * KT:(kt + 1) * KT],
                            qT[n][hs:hs + DK, qs * SW + a:qs * SW + b],
                            start=True, stop=True)
                    et = expp.tile([128, 2, SW], bf16, tag="exp", name=f"e{n}{qs}{kt}")
                    nc.scalar.activation(
                        et[:, :, a:b],
                        ps.rearrange("p (h c) -> p h c", h=2)[:, :, a:b], EXP)
                    if cls_[qs][kt] == 2:
                        pi = pidx[qs][kt]
                        for hp in range(HPC):
                            nc.vector.tensor_tensor(
                                et[:, hp, a:b], et[:, hp, a:b],
                                mpat_sb[:, pi, a:b], MUL)
                    exp_tiles[kt] = et

                def emit_av(kt):
                    # AV chain step: the two head chains interleaved so
                    # consecutive matmuls alternate PSUM banks
                    i = kts.index(kt)
                    a, b = span_[qs][kt]
                    for hp in range(HPC):
                        nc.tensor.matmul(
                            avs[hp][:, a:b],
                            vaug[n][:, kt * VW + hp * 65:kt * VW + (hp + 1) * 65],
                            exp_tiles[kt][:, hp, a:b],
                            start=(i == 0), stop=(i == len(kts) - 1))

                # software pipeline in 2-kt groups: AV lags scores/exp by one
                # group, so the post-exp serial tail is just the last group
                groups = [kts[g:g + 2] for g in range(0, len(kts), 2)]
                prev = None
                for group in groups:
                    for kt in group:
                        emit_scores(kt)
                    if prev is not None:
                        for kt in prev:
                            emit_av(kt)
                    prev = group
                for kt in prev:
                    emit_av(kt)
                for hp in range(HPC):
                    av = avs[hp]
                    den = nrm.tile([65, SW], bf16, tag="den", name=f"dn{n}{qs}{hp}")
                    nc.vector.tensor_copy(den[64:65, :], av[64:65, :])
                    bc = sp.tile([DK, SW], f32, tag="sp", name=f"bc{n}{qs}{hp}")
                    nc.tensor.matmul(bc[:], ones65[64:65, :], den[64:65, :],
                                     start=True, stop=True)
                    rec = nrm.tile([DK, SW], f32, tag="rec", name=f"rc{n}{qs}{hp}")
                    nc.vector.reciprocal_approx_fast(rec[:], bc[:])
                    nc.vector.tensor_tensor(
                        headT[n][hp][:, qs * SW:(qs + 1) * SW],
                        av[:DK, :], rec[:], MUL)

            def emit_a2a(n):
                for hp in range(HPC):
                    nc.sync.dma_start(
                        a2a_in[n][:, hp * DK:(hp + 1) * DK, :].transpose([1, 0, 2]),
                        headT[n][hp][:, :].rearrange("p (j c) -> p j c", j=NC))
                nc.gpsimd.collective_compute(
                    "AllToAll", mybir.AluOpType.bypass,
                    replica_groups=[list(range(NC))],
                    ins=[a2a_in[n].opt()], outs=[a2a_out[n].opt()])

            def emit_wo(n):
                rhs_t = wos.tile([128, NC, CB], bf16, tag="rhs", name=f"rhs{n}")
                nc.sync.dma_start(rhs_t[:], a2a_out[n][:, :, :].transpose([1, 0, 2]))
                for mts in ((0, 1, 2), (3, 4, 5), (6, 7)):
                    pss = [sp.tile([128, 1024], f32, tag="sp", name=f"wp{n}{mt}")
                           for mt in mts]
                    for jj in range(8):
                        for k, mt in enumerate(mts):
                            nc.tensor.matmul(pss[k][:, :CB],
                                             wo_sb[:, jj, mt * KT:(mt + 1) * KT],
                                             rhs_t[:, jj, :],
                                             start=(jj == 0), stop=(jj == 7))
                    for k, mt in enumerate(mts):
                        ob = osb.tile([128, CB], f32, tag="osb", name=f"ob{n}{mt}")
                        nc.vector.tensor_scalar_add(ob[:], pss[k][:, :CB], bo_sb[:, mt, :])
                        nc.scalar.dma_start(out_t[mt * KT:(mt + 1) * KT, n * CB:(n + 1) * CB],
                                            ob[:])

            # ---- pipeline: attention interleaved between projection blocks
            # so PSUM-ring rotation matches data readiness ----
            emit_inputs(0)
            emit_inputs(1)
            emit_proj_block(0, 0)
            emit_proj_block(0, 1)
            emit_attn_qs(0, 0)
            emit_proj_block(0, 2)
            emit_attn_qs(0, 1)
            emit_proj_block(0, 3)
            emit_attn_qs(0, 2)
            emit_proj_block(1, 0)
            emit_attn_qs(0, 3)
            nc.scalar.dma_start(wo_sb[:], wo[:, :, :])
            emit_a2a(0)
            emit_proj_block(1, 1)
            emit_attn_qs(1, 0)
            emit_proj_block(1, 2)
            emit_attn_qs(1, 1)
            emit_proj_block(1, 3)
            emit_attn_qs(1, 2)
            emit_attn_qs(1, 3)
            emit_a2a(1)
            emit_wo(0)
            emit_wo(1)

    nc.compile()
    return nc


def kernel(x, y, mask, Wq, bq, Wk, bk, Wv, bv, Wo, bo, _trace=False):
    x = np.asarray(x, np.float32)
    y = np.asarray(y, np.float32)
    cls_, span_, pidx, pats = _classify_blocks(mask)

    key = (x.shape,
           tuple(tuple(c) for c in cls_),
           tuple(tuple(s) for s in span_),
           tuple(tuple(p) for p in pidx),
           pats.tobytes())
    if key not in _CACHE:
        _CACHE[key] = _build(cls_, span_, pidx, pats.shape[0])
    nc = _CACHE[key]

    fac = np.float32(1.0 / np.sqrt(DK))
    xtb = np.ascontiguousarray(
        np.concatenate([x[n].T for n in range(NB)], axis=1)).astype(BF16)
    ytb = np.ascontiguousarray(
        np.concatenate([y[n].T for n in range(NB)], axis=1)).astype(BF16)
    Wq32 = np.asarray(Wq, np.float32) * fac
    bq32 = np.asarray(bq, np.float32) * fac

    def pmajor(w):
        # [DM, X] -> [128, 8, X] with [p, t, :] = w[t*128+p, :]
        w = np.asarray(w)
        return np.ascontiguousarray(w.reshape(8, 128, w.shape[1]).transpose(1, 0, 2))

    wo_pm = pmajor(np.asarray(Wo, np.float32)).astype(BF16)
    bo_pm = pmajor(np.asarray(bo, np.float32).reshape(DM, 1))
    mpat_t = np.ascontiguousarray(pats.transpose(1, 0, 2))

    in_maps = []
    for c in range(NC):
        d0 = c * DPC
        wv_aug = np.zeros((DM, VW), np.float32)
        bv1 = np.zeros((1, VW), np.float32)
        for hp in range(HPC):
            h = HPC * c + hp
            wv_aug[:, hp * 65:hp * 65 + DK] = np.asarray(Wv, np.float32)[:, h * DK:(h + 1) * DK]
            bv1[0, hp * 65:hp * 65 + DK] = np.asarray(bv, np.float32)[h * DK:(h + 1) * DK]
            bv1[0, hp * 65 + DK] = 1.0
        in_maps.append({
            "xtb": xtb, "ytb": ytb,
            "wq": pmajor(Wq32[:, d0:d0 + DPC]).astype(BF16),
            "wk": pmajor(np.asarray(Wk, np.float32)[:, d0:d0 + DPC]).astype(BF16),
            "wv": pmajor(wv_aug).astype(BF16),
            "wo": wo_pm,
            "bq": bq32[d0:d0 + DPC].reshape(DPC, 1),
            "bk": np.asarray(bk, np.float32)[d0:d0 + DPC].reshape(DPC, 1),
            "bv1": bv1.astype(BF16),
            "bo": bo_pm,
            "mpat": mpat_t,
        })

    res = run_bass_kernel_spmd(nc, in_maps, core_ids=list(range(NC)), trace=_trace)
    out = np.empty((NB, L, DM), np.float32)
    for c in range(NC):
        for n in range(NB):
            out[n, c * CB:(c + 1) * CB, :] = res.results[c]["out_t"][:, n * CB:(n + 1) * CB].T
    if _trace:
        kernel.last_results = res
    return out


# revision 27
# speedup vs baseline: 1.0599x; 1.0599x over previous
"""Multi-head attention (N=2, L=2048, 16 heads x 64) on 8 TRN2 NeuronCores.

Head-parallel attention (2 heads/core) with a per-batch software pipeline:
attention emission interleaved between projection blocks (so the shared
PSUM ring rotates in data-readiness order), per-batch AllToAll (head-split
-> sequence-split) with collective #0 hidden under batch-1 compute and the
batch-0 output projection hidden under collective #1.

Key scheduling choices:
- Few, large DMAs: weights host-pre-shuffled to partition-major [128,t,d]
  so each const load is one contiguous chunk per partition; input quarters
  loaded as [128,8,512] tiles; issue split across both HWDGE rings.
- Same-bank PSUM accumulation stalls (~+100ns/matmul) avoided by
  interleaving independent chain pairs (K/V and V/V projection chains, the
  two heads' AV chains, output-projection mt-groups) across banks.
- Softmax normalization without DMA round trips: ones-column in V gives the
  denominator as AV row 64; K=1 matmul broadcasts it, reciprocal_approx_fast
  (5x faster than DVE reciprocal) + one DVE multiply normalize.
- exp is one ScalarE call per k-tile covering both heads via a 3D AP.
"""
import sys

sys.path.insert(0, "/opt/trn_rl_repo")

import numpy as np
import ml_dtypes

import concourse.bass as bass
import concourse.bacc as bacc
import concourse.mybir as mybir
import concourse.tile as tile
from concourse.bass_utils import run_bass_kernel_spmd

BF16 = ml_dtypes.bfloat16

DM = 1024      # dmodel
DK = 64        # head dim
H = 16         # heads
NB = 2         # batch
L = 2048       # seq len
R = NB * L
NC = 8         # cores
HPC = H // NC  # heads per core = 2
DPC = HPC * DK  # depth per core = 128

SW = 512       # q sub-window
KT = 128       # k tile
NQS = L // SW   # 4 q blocks per batch
NKT = L // KT   # 16 k tiles per batch
CB = L // NC    # 256: per-batch per-core output chunk
VW = 65 * HPC   # 130: augmented v width (both heads, +ones col each)

_CACHE = {}


def _classify_blocks(mask):
    """Per (qs, kt) block: 0=skip, 1=full, 2=partial (+ q-span, pattern)."""
    mask = np.asarray(mask, dtype=bool)
    cls = [[0] * NKT for _ in range(NQS)]
    span = [[None] * NKT for _ in range(NQS)]
    pat_ids = {}
    pats = []
    pat_idx = [[-1] * NKT for _ in range(NQS)]
    for qs in range(NQS):
        for kt in range(NKT):
            sub = mask[qs * SW:(qs + 1) * SW, kt * KT:(kt + 1) * KT]
            rows = np.nonzero(sub.any(axis=1))[0]
            if rows.size == 0:
                cls[qs][kt] = 0
            elif sub.all():
                cls[qs][kt] = 1
                span[qs][kt] = (0, SW)
            else:
                cls[qs][kt] = 2
                span[qs][kt] = (int(rows[0]), int(rows[-1]) + 1)
                pat = np.ascontiguousarray(sub.T).astype(BF16)  # [128 k, SW q]
                key = pat.tobytes()
                if key not in pat_ids:
                    pat_ids[key] = len(pats)
                    pats.append(pat)
                pat_idx[qs][kt] = pat_ids[key]
    # the first included kt of each sub-window must cover the full 512
    # columns (its start=True matmul clears PSUM has_written)
    for qs in range(NQS):
        for kt in range(NKT):
            if cls[qs][kt]:
                span[qs][kt] = (0, SW)
                break
    if not pats:
        pats.append(np.ones((KT, SW), dtype=BF16))
    return cls, span, pat_idx, np.stack(pats)


def _build(cls_, span_, pidx, n_pat):
    nc = bacc.Bacc("TRN2", target_bir_lowering=False, debug=False,
                   enable_asserts=False, num_devices=NC)
    f32, bf16 = mybir.dt.float32, mybir.dt.bfloat16
    EXP = mybir.ActivationFunctionType.Exp
    MUL = mybir.AluOpType.mult

    # weights arrive host-pre-shuffled into partition-major layouts so every
    # const DMA is one contiguous chunk per partition (few, large descriptors)
    xtb = nc.dram_tensor("xtb", [DM, R], bf16, kind="ExternalInput")
    ytb = nc.dram_tensor("ytb", [DM, R], bf16, kind="ExternalInput")
    wq = nc.dram_tensor("wq", [128, 8, DPC], bf16, kind="ExternalInput")
    wk = nc.dram_tensor("wk", [128, 8, DPC], bf16, kind="ExternalInput")
    wv = nc.dram_tensor("wv", [128, 8, VW], bf16, kind="ExternalInput")
    wo = nc.dram_tensor("wo", [128, 8, DM], bf16, kind="ExternalInput")
    bqd = nc.dram_tensor("bq", [DPC, 1], f32, kind="ExternalInput")
    bkd = nc.dram_tensor("bk", [DPC, 1], f32, kind="ExternalInput")
    bv1 = nc.dram_tensor("bv1", [1, VW], bf16, kind="ExternalInput")
    bod = nc.dram_tensor("bo", [128, 8, 1], f32, kind="ExternalInput")
    mpat = nc.dram_tensor("mpat", [KT, n_pat, SW], bf16, kind="ExternalInput")
    out_t = nc.dram_tensor("out_t", [DM, NB * CB], f32, kind="ExternalOutput")

    with tile.TileContext(nc) as tc:
        with (
            tc.tile_pool(name="cst", bufs=1) as cst,
            tc.tile_pool(name="xy", bufs=6) as xy,
            tc.tile_pool(name="big", bufs=1) as big,
            tc.tile_pool(name="expp", bufs=12) as expp,
            tc.tile_pool(name="nrm", bufs=3) as nrm,
            tc.tile_pool(name="wos", bufs=2) as wos,
            tc.tile_pool(name="osb", bufs=3) as osb,
            tc.tile_pool(name="sp", bufs=3, space="PSUM") as sp,
            tc.tile_pool(name="avp", bufs=2, space="PSUM") as avp,
            tc.tile_pool(name="dram", bufs=1, space="DRAM") as dram,
        ):
            # ---- constants (scalar HWDGE ring; contiguous partition-major) ----
            bq_sb = cst.tile([DPC, 1], f32)
            bk_sb = cst.tile([DPC, 1], f32)
            bv1_sb = cst.tile([1, VW], bf16)
            bo_sb = cst.tile([128, 8, 1], f32)
            nc.scalar.dma_start(bk_sb[:], bkd[:])
            nc.scalar.dma_start(bq_sb[:], bqd[:])
            nc.scalar.dma_start(bv1_sb[:], bv1[:])
            nc.scalar.dma_start(bo_sb[:], bod[:, :, :])
            wq_sb = cst.tile([128, 8, DPC], bf16)
            wk_sb = cst.tile([128, 8, DPC], bf16)
            wv_sb = cst.tile([128, 8, VW], bf16)
            wo_sb = cst.tile([128, 8, DM], bf16)
            nc.scalar.dma_start(wk_sb[:], wk[:, :, :])
            nc.scalar.dma_start(wv_sb[:], wv[:, :, :])
            nc.scalar.dma_start(wq_sb[:], wq[:, :, :])
            mpat_sb = cst.tile([KT, n_pat, SW], bf16)
            nc.scalar.dma_start(mpat_sb[:], mpat[:, :, :])
            ones_row = cst.tile([1, 128], bf16)
            nc.vector.memset(ones_row[:], 1.0)
            ones65 = cst.tile([65, DK], bf16)
            nc.vector.memset(ones65[:], 1.0)

            # preload the exp table set during the DMA phase
            bar_sb = cst.tile([1, 8], f32)
            nc.vector.memset(bar_sb[:], 0.0)
            dum = cst.tile([1, 8], f32)
            nc.scalar.activation(dum[:], bar_sb[:], EXP)

            # ---- start-of-kernel barrier (absorbs launch skew) ----
            bar_in = dram.tile([1, 8], f32, tag="bar_in")
            bar_out = dram.tile([1, 8], f32, tag="bar_out")
            nc.sync.dma_start(bar_in[:], bar_sb[:])
            nc.gpsimd.collective_compute(
                "AllReduce", mybir.AluOpType.add,
                replica_groups=[list(range(NC))],
                ins=[bar_in.opt()], outs=[bar_out.opt()])

            qT = [big.tile([DPC, L], bf16, tag=f"qT{n}", name=f"qT{n}") for n in range(NB)]
            kT = [big.tile([DPC, L], bf16, tag=f"kT{n}", name=f"kT{n}") for n in range(NB)]
            vaug = [big.tile([128, NKT * VW], bf16, tag=f"va{n}", name=f"va{n}") for n in range(NB)]
            headT = [[big.tile([DK, L], bf16, tag=f"hT{n}{hp}", name=f"hT{n}{hp}")
                      for hp in range(HPC)] for n in range(NB)]

            a2a_in = [dram.tile([NC, DPC, CB], bf16, tag=f"a2ai{n}", name=f"a2ai{n}")
                      for n in range(NB)]
            a2a_out = [dram.tile([NC, DPC, CB], bf16, tag=f"a2ao{n}", name=f"a2ao{n}")
                       for n in range(NB)]

            # input tiles: [128, 4 dm-chunks, 1024 cols] so each DMA
            # descriptor is a contiguous 2KB run (descriptor-rate limited
            # at 1KB); indexed [batch][col-half][dm-half]
            ytile = [[[None] * 2 for _ in range(2)] for _ in range(NB)]
            xtile = [[[None] * 2 for _ in range(2)] for _ in range(NB)]

            def emit_inputs(n):
                for ch in range(2):
                    for src, tiles, nm in ((ytb, ytile, "y"), (xtb, xtile, "x")):
                        for hf in range(2):
                            t = xy.tile([128, 4, 2 * SW], bf16, tag="xy",
                                        name=f"{nm}{n}c{ch}h{hf}")
                            nc.sync.dma_start(
                                t[:], src[hf * 512:(hf + 1) * 512,
                                          n * L + ch * 1024:n * L + (ch + 1) * 1024]
                                .rearrange("(t p) c -> p t c", p=128))
                            tiles[n][ch][hf] = t

            def ysl(n, b, dt, c0, c1):
                return ytile[n][b // 2][dt // 4][:, dt % 4,
                                                 (b % 2) * SW + c0:(b % 2) * SW + c1]

            def xsl(n, b, dt, c0, c1):
                return xtile[n][b // 2][dt // 4][:, dt % 4,
                                                 (b % 2) * SW + c0:(b % 2) * SW + c1]

            def _v_chain(n, b, j, psv, dt, last):
                # one step of a V-projection chain (stationary = y k-tile)
                if dt < 8:
                    nc.tensor.matmul(psv[:, :VW], ysl(n, b, dt, j * KT, (j + 1) * KT),
                                     wv_sb[:, dt, :], start=(dt == 0), stop=False)
                else:
                    nc.tensor.matmul(psv[:, :VW], ones_row[:], bv1_sb[:],
                                     start=False, stop=True)

            def emit_proj_block(n, b):
                # chains interleaved in PAIRS so consecutive matmuls hit
                # alternating PSUM banks (avoids same-bank drain stalls)
                # pair 1: K chain & V chain j=0
                ps_k = sp.tile([128, 1024], f32, tag="sp", name=f"kp{n}{b}")
                psv0 = sp.tile([128, 1024], f32, tag="sp", name=f"vp{n}{b}0")
                for dt in range(9):
                    if dt < 8:
                        nc.tensor.matmul(ps_k[:DPC, :SW], wk_sb[:, dt, :],
                                         ysl(n, b, dt, 0, SW),
                                         start=(dt == 0), stop=(dt == 7))
                    _v_chain(n, b, 0, psv0, dt, False)
                nc.vector.tensor_scalar_add(kT[n][:, b * SW:(b + 1) * SW],
                                            ps_k[:DPC, :SW], bk_sb[:])
                nc.vector.tensor_copy(vaug[n][:, (b * 4) * VW:(b * 4 + 1) * VW],
                                      psv0[:, :VW])
                # pair 2: V chains j=1 & j=2
                psv1 = sp.tile([128, 1024], f32, tag="sp", name=f"vp{n}{b}1")
                psv2 = sp.tile([128, 1024], f32, tag="sp", name=f"vp{n}{b}2")
                for dt in range(9):
                    _v_chain(n, b, 1, psv1, dt, False)
                    _v_chain(n, b, 2, psv2, dt, False)
                nc.vector.tensor_copy(vaug[n][:, (b * 4 + 1) * VW:(b * 4 + 2) * VW],
                                      psv1[:, :VW])
                nc.vector.tensor_copy(vaug[n][:, (b * 4 + 2) * VW:(b * 4 + 3) * VW],
                                      psv2[:, :VW])
                # pair 3: V chain j=3 & Q chain
                psv3 = sp.tile([128, 1024], f32, tag="sp", name=f"vp{n}{b}3")
                ps_q = sp.tile([128, 1024], f32, tag="sp", name=f"qp{n}{b}")
                for dt in range(9):
                    _v_chain(n, b, 3, psv3, dt, False)
                    if dt < 8:
                        nc.tensor.matmul(ps_q[:DPC, :SW], wq_sb[:, dt, :],
                                         xsl(n, b, dt, 0, SW),
                                         start=(dt == 0), stop=(dt == 7))
                nc.vector.tensor_copy(vaug[n][:, (b * 4 + 3) * VW:(b * 4 + 4) * VW],
                                      psv3[:, :VW])
                nc.vector.tensor_scalar_add(qT[n][:, b * SW:(b + 1) * SW],
                                            ps_q[:DPC, :SW], bq_sb[:])

            def emit_attn_qs(n, qs):
                kts = [kt for kt in range(NKT) if cls_[qs][kt]]
                exp_tiles = {}
                avs = [avp.tile([65, SW], f32, tag="avp", name=f"av{n}{qs}{hp}")
                       for hp in range(HPC)]

                def emit_scores(kt):
                    a, b = span_[qs][kt]
                    ps = sp.tile([128, 1024], f32, tag="sp", name=f"s{n}{qs}{kt}")
                    for hp in range(HPC):
                        hs = hp * DK
                        nc.tensor.matmul(
                            ps[:KT, hp * SW + a:hp * SW + b],
                            kT[n][hs:hs + DK, kt * KT:(kt + 1) * KT],
                            qT[n][hs:hs + DK, qs * SW + a:qs * SW + b],
                            start=True, stop=True)
                    et = expp.tile([128, 2, SW], bf16, tag="exp", name=f"e{n}{qs}{kt}")
                    nc.scalar.activation(
                        et[:, :, a:b],
                        ps.rearrange("p (h c) -> p h c", h=2)[:, :, a:b], EXP)
                    if cls_[qs][kt] == 2:
                        pi = pidx[qs][kt]
                        for hp in range(HPC):
                            nc.vector.tensor_tensor(
                                et[:, hp, a:b], et[:, hp, a:b],
                                mpat_sb[:, pi, a:b], MUL)
                    exp_tiles[kt] = et

                def emit_av(kt):
                    # AV chain step: the two head chains interleaved so
                    # consecutive matmuls alternate PSUM banks
                    i = kts.index(kt)
                    a, b = span_[qs][kt]
                    for hp in range(HPC):
                        nc.tensor.matmul(
                            avs[hp][:, a:b],
                            vaug[n][:, kt * VW + hp * 65:kt * VW + (hp + 1) * 65],
                            exp_tiles[kt][:, hp, a:b],
                            start=(i == 0), stop=(i == len(kts) - 1))

                # software pipeline in 2-kt groups: AV lags scores/exp by one
                # group, so the post-exp serial tail is just the last group
                groups = [kts[g:g + 2] for g in range(0, len(kts), 2)]
                prev = None
                for group in groups:
                    for kt in group:
                        emit_scores(kt)
                    if prev is not None:
                        for kt in prev:
                            emit_av(kt)
                    prev = group
                for kt in prev:
                    emit_av(kt)
                for hp in range(HPC):
                    av = avs[hp]
                    den = nrm.tile([65, SW], bf16, tag="den", name=f"dn{n}{qs}{hp}")
                    nc.vector.tensor_copy(den[64:65, :], av[64:65, :])
                    bc = sp.tile([DK, SW], f32, tag="sp", name=f"bc{n}{qs}{hp}")
                    nc.tensor.matmul(bc[:], ones65[64:65, :], den[64:65, :],
                                     start=True, stop=True)
                    rec = nrm.tile([DK, SW], f32, tag="rec", name=f"rc{n}{qs}{hp}")
                    nc.vector.reciprocal_approx_fast(rec[:], bc[:])
                    nc.vector.tensor_tensor(
                        headT[n][hp][:, qs * SW:(qs + 1) * SW],
                        av[:DK, :], rec[:], MUL)

            def emit_a2a(n):
                for hp in range(HPC):
                    nc.sync.dma_start(
                        a2a_in[n][:, hp * DK:(hp + 1) * DK, :].transpose([1, 0, 2]),
                        headT[n][hp][:, :].rearrange("p (j c) -> p j c", j=NC))
                nc.gpsimd.collective_compute(
                    "AllToAll", mybir.AluOpType.bypass,
                    replica_groups=[list(range(NC))],
                    ins=[a2a_in[n].opt()], outs=[a2a_out[n].opt()])

            def emit_wo(n):
                rhs_t = wos.tile([128, NC, CB], bf16, tag="rhs", name=f"rhs{n}")
                nc.sync.dma_start(rhs_t[:], a2a_out[n][:, :, :].transpose([1, 0, 2]))
                for mts in ((0, 1, 2), (3, 4, 5), (6, 7)):
                    pss = [sp.tile([128, 1024], f32, tag="sp", name=f"wp{n}{mt}")
                           for mt in mts]
                    for jj in range(8):
                        for k, mt in enumerate(mts):
                            nc.tensor.matmul(pss[k][:, :CB],
                                             wo_sb[:, jj, mt * KT:(mt + 1) * KT],
                                             rhs_t[:, jj, :],
                                             start=(jj == 0), stop=(jj == 7))
                    for k, mt in enumerate(mts):
                        ob = osb.tile([128, CB], f32, tag="osb", name=f"ob{n}{mt}")
                        nc.vector.tensor_scalar_add(ob[:], pss[k][:, :CB], bo_sb[:, mt, :])
                        nc.scalar.dma_start(out_t[mt * KT:(mt + 1) * KT, n * CB:(n + 1) * CB],
                                            ob[:])

            # ---- pipeline: attention interleaved between projection blocks
            # so PSUM-ring rotation matches data readiness ----
            emit_inputs(0)
            emit_inputs(1)
            emit_proj_block(0, 0)
            emit_proj_block(0, 1)
            emit_attn_qs(0, 0)
            emit_proj_block(0, 2)
            emit_attn_qs(0, 1)
            emit_proj_block(0, 3)
            emit_attn_qs(0, 2)
            emit_proj_block(1, 0)
            emit_attn_qs(0, 3)
            nc.scalar.dma_start(wo_sb[:], wo[:, :, :])
            emit_a2a(0)
            emit_proj_block(1, 1)
            emit_attn_qs(1, 0)
            emit_proj_block(1, 2)
            emit_attn_qs(1, 1)
            emit_proj_block(1, 3)
            emit_attn_qs(1, 2)
            emit_attn_qs(1, 3)
            emit_a2a(1)
            emit_wo(0)
            emit_wo(1)

    nc.compile()
    return nc


def kernel(x, y, mask, Wq, bq, Wk, bk, Wv, bv, Wo, bo, _trace=False):
    x = np.asarray(x, np.float32)
    y = np.asarray(y, np.float32)
    cls_, span_, pidx, pats = _classify_blocks(mask)

    key = (x.shape,
           tuple(tuple(c) for c in cls_),
           tuple(tuple(s) for s in span_),
           tuple(tuple(p) for p in pidx),
           pats.tobytes())
    if key not in _CACHE:
        _CACHE[key] = _build(cls_, span_, pidx, pats.shape[0])
    nc = _CACHE[key]

    fac = np.float32(1.0 / np.sqrt(DK))
    xtb = np.ascontiguousarray(
        np.concatenate([x[n].T for n in range(NB)], axis=1)).astype(BF16)
    ytb = np.ascontiguousarray(
        np.concatenate([y[n].T for n in range(NB)], axis=1)).astype(BF16)
    Wq32 = np.asarray(Wq, np.float32) * fac
    bq32 = np.asarray(bq, np.float32) * fac

    def pmajor(w):
        # [DM, X] -> [128, 8, X] with [p, t, :] = w[t*128+p, :]
        w = np.asarray(w)
        return np.ascontiguousarray(w.reshape(8, 128, w.shape[1]).transpose(1, 0, 2))

    wo_pm = pmajor(np.asarray(Wo, np.float32)).astype(BF16)
    bo_pm = pmajor(np.asarray(bo, np.float32).reshape(DM, 1))
    mpat_t = np.ascontiguousarray(pats.transpose(1, 0, 2))

    in_maps = []
    for c in range(NC):
        d0 = c * DPC
        wv_aug = np.zeros((DM, VW), np.float32)
        bv1 = np.zeros((1, VW), np.float32)
        for hp in range(HPC):
            h = HPC * c + hp
            wv_aug[:, hp * 65:hp * 65 + DK] = np.asarray(Wv, np.float32)[:, h * DK:(h + 1) * DK]
            bv1[0, hp * 65:hp * 65 + DK] = np.asarray(bv, np.float32)[h * DK:(h + 1) * DK]
            bv1[0, hp * 65 + DK] = 1.0
        in_maps.append({
            "xtb": xtb, "ytb": ytb,
            "wq": pmajor(Wq32[:, d0:d0 + DPC]).astype(BF16),
            "wk": pmajor(np.asarray(Wk, np.float32)[:, d0:d0 + DPC]).astype(BF16),
            "wv": pmajor(wv_aug).astype(BF16),
            "wo": wo_pm,
            "bq": bq32[d0:d0 + DPC].reshape(DPC, 1),
            "bk": np.asarray(bk, np.float32)[d0:d0 + DPC].reshape(DPC, 1),
            "bv1": bv1.astype(BF16),
            "bo": bo_pm,
            "mpat": mpat_t,
        })

    res = run_bass_kernel_spmd(nc, in_maps, core_ids=list(range(NC)), trace=_trace)
    out = np.empty((NB, L, DM), np.float32)
    for c in range(NC):
        for n in range(NB):
            out[n, c * CB:(c + 1) * CB, :] = res.results[c]["out_t"][:, n * CB:(n + 1) * CB].T
    if _trace:
        kernel.last_results = res
    return out
